# revision 28
# baseline (speedup 1.0000x reference)
"""Trainium2 Bass kernel for nn_CustomMultiHeadAttention (B2 T2048 D1024 H16).

Sharding: 8 cores = 2 batches x 4 head-groups (4 heads/core, tensor-parallel
columns for Wq/Wk/Wv, rows for Wo; host sums the 4 row-parallel partials).

Key algebraic simplification: F_ij = bs*(fj-fi)/(fi*fj+eps) ~= bs*(1/fi-1/fj)
(rank-1, since eps << fi*fj for the given inputs). The row term bs/fi is
constant per softmax row and cancels; only a per-column bias c_j = -bs/fj
survives. c_j is added to the QK^T logits via a K=1 ones-matmul accumulated
into the same PSUM (4 concurrent row-group-tiled adds), replacing the former
per-tile F-matrix identity matmuls and the [B,T,T] F operand entirely.

Per-core pipeline:
  x^T streamed -> Q^T/K^T/V^T projections (PE) -> V^T PE-transposed to V[t,c]
  -> per i-chunk: S = QK^T (K=64 row-tiled head pairs) + c_j rank-1 PSUM add
  -> ACT exp (constant upper-bound shift, no row-max reduce; accum_out = row
  sums) -> bf16 probs normalized (DVE) -> DMA-xbar transpose -> P^T @ V
  (col-tiled head pairs) -> out-proj partial.
"""

from contextlib import ExitStack

import numpy as np
import ml_dtypes

import concourse.bass as bass
import concourse.mybir as mybir
import concourse.tile as tile
from concourse import bacc
from concourse.bass_utils import run_bass_kernel_spmd
from concourse.masks import make_identity

AF = mybir.ActivationFunctionType
ALU = mybir.AluOpType
F32 = mybir.dt.float32
R32 = mybir.dt.float32r
BF16 = mybir.dt.bfloat16

X = mybir.AxisListType.X

B, T, D = 2, 2048, 1024
H, DH = 16, 64
H_LOC = 4
C_LOC = H_LOC * DH          # 256
N_CORES = 8
SCALE = DH ** -0.5
EPS = 1e-8
P = 128
ICH, JCH, KCH = T // P, T // P, D // P   # 16, 16, 8
SL = 4
IC_PER_SL = ICH // SL       # 4


def _build_program(maxf: float):
    nc = bacc.Bacc("TRN2", target_bir_lowering=False, debug=False,
                   num_devices=N_CORES)

    xq_d = nc.dram_tensor("xq", [D, T], BF16, kind="ExternalInput").ap()
    xk_d = nc.dram_tensor("xk", [D, T], BF16, kind="ExternalInput").ap()
    xv_d = nc.dram_tensor("xv", [D, T], BF16, kind="ExternalInput").ap()
    wq_d = nc.dram_tensor("wq", [D, C_LOC], BF16, kind="ExternalInput").ap()
    wk_d = nc.dram_tensor("wk", [D, C_LOC], BF16, kind="ExternalInput").ap()
    wv_d = nc.dram_tensor("wv", [D, C_LOC], BF16, kind="ExternalInput").ap()
    wo_d = nc.dram_tensor("wo", [C_LOC, D], BF16, kind="ExternalInput").ap()
    c_d = nc.dram_tensor("cvec", [P, T], BF16, kind="ExternalInput").ap()
    out_d = nc.dram_tensor("out", [T, D], F32, kind="ExternalOutput").ap()

    with tile.TileContext(nc) as tc, ExitStack() as ctx:
        const = ctx.enter_context(tc.tile_pool(name="const", bufs=1))
        wpool = ctx.enter_context(tc.tile_pool(name="w", bufs=1))
        qkv = ctx.enter_context(tc.tile_pool(name="qkv", bufs=1))
        xpool = ctx.enter_context(tc.tile_pool(name="x", bufs=2))
        phpool = ctx.enter_context(tc.tile_pool(name="ph", bufs=4))
        ptpool = ctx.enter_context(tc.tile_pool(name="pt", bufs=1))
        stats = ctx.enter_context(tc.tile_pool(name="stats", bufs=1))
        opool = ctx.enter_context(tc.tile_pool(name="o", bufs=2))
        psum = ctx.enter_context(tc.tile_pool(name="ps", bufs=3, space="PSUM"))
        pvps = ctx.enter_context(tc.tile_pool(name="pv", bufs=2, space="PSUM"))

        identb = const.tile([P, P], BF16)
        make_identity(nc, identb)
        onescol = const.tile([P, 1], BF16)
        nc.any.memset(onescol[:], 1.0)
        onesrow = const.tile([1, P], F32)
        nc.any.memset(onesrow[:], 1.0)
        neg_a = const.tile([P, 1], F32)

        wq_s = wpool.tile([P, KCH, C_LOC], BF16, tag="wq")
        nc.sync.dma_start(wq_s[:], wq_d.rearrange("(kc p) c -> p kc c", p=P))
        wk_s = wpool.tile([P, KCH, C_LOC], BF16, tag="wk")
        nc.sync.dma_start(wk_s[:], wk_d.rearrange("(kc p) c -> p kc c", p=P))
        wv_s = wpool.tile([P, KCH, C_LOC], BF16, tag="wv")
        nc.sync.dma_start(wv_s[:], wv_d.rearrange("(kc p) c -> p kc c", p=P))
        wo_s = wpool.tile([P, 2, D], BF16, tag="wo")
        nc.sync.dma_start(wo_s[:], wo_d.rearrange("(cc p) o -> p cc o", p=P))

        # ---- projections: dst[c % 128, pair, t] = (W.T x^T)  ----
        qt_s = qkv.tile([P, 2, T], BF16, tag="qt")
        kt_s = qkv.tile([P, 2, T], BF16, tag="kt")
        vt_s = qkv.tile([P, 2, T], BF16, tag="vt")
        # Augmented per-head Q/K: rows 0:63 = head channels at base
        # partition 0, row 64 = ones (Q) / the c_j column-bias row (K).
        # One K=65 matmul then yields S + c in a single 512-cycle pass.
        qa_s = [qkv.tile([65, T], BF16, tag=f"qa{h}", name=f"qa{h}")
                for h in range(H_LOC)]
        ka_s = [qkv.tile([65, T], BF16, tag=f"ka{h}", name=f"ka{h}")
                for h in range(H_LOC)]
        def _proj(x_d, w_s, dst):
            for th in range(2):                      # halves of T
                t0 = th * 1024
                pstiles = [psum.tile([P, 1024], F32, tag="ps", name=f"pj{th}{pi}")
                           for pi in range(2)]
                for kc in range(KCH):
                    xt = xpool.tile([P, 1024], BF16, tag="x")
                    nc.gpsimd.dma_start(
                        xt[:], x_d[kc * P:(kc + 1) * P, t0:t0 + 1024])
                    for pair in range(2):
                        lhsT = w_s[:, kc, pair * P:(pair + 1) * P]
                        for nb in range(2):
                            nc.tensor.matmul(
                                pstiles[pair][:, nb * 512:(nb + 1) * 512],
                                lhsT, xt[:, nb * 512:(nb + 1) * 512],
                                start=(kc == 0), stop=(kc == KCH - 1))
                for pair in range(2):
                    nc.vector.tensor_copy(dst[:, pair, t0:t0 + 1024],
                                          pstiles[pair][:])

        # ---- V^T -> V[t % 128, tc, c] bf16 via PE transpose ----
        v_s = qkv.tile([P, ICH, C_LOC], BF16, tag="v")

        def _v_transpose():
          for tc_i in range(ICH):
            for pair in range(2):
                tp = pvps.tile([P, 512], BF16, tag="pv", name=f"tp{tc_i}_{pair}")
                nc.tensor.transpose(
                    tp[:, 0:P], vt_s[:, pair, tc_i * P:(tc_i + 1) * P],
                    identb[:])
                nc.vector.tensor_copy(
                    v_s[:, tc_i, pair * P:(pair + 1) * P], tp[:, 0:P])
          return

        # ---- exp shift bound: A = (S/2)(max qsq + max ksq) + S*margin ----
        # qsq here sums both heads of a pair (K=128 ones reduce) -- a
        # slightly looser but still valid upper bound.
        gmax = stats.tile([1, 16], F32, tag="gmax")

        def _bounds_inner(qi, src):
            for pair in range(2):
                sq = xpool.tile([P, T], BF16, tag="x", name=f"sq{qi}{pair}")
                nc.vector.tensor_mul(sq[:], src[:, pair, :], src[:, pair, :])
                for nb in range(4):
                    bp = psum.tile([P, 1024], F32, tag="ps",
                                   name=f"bp{qi}{pair}{nb}")
                    nc.tensor.matmul(
                        bp[0:1, 0:512], onescol[:],
                        sq[:, nb * 512:(nb + 1) * 512],
                        start=True, stop=True)
                    idx = qi * 8 + pair * 4 + nb
                    nc.vector.reduce_max(gmax[0:1, idx:idx + 1],
                                         bp[0:1, 0:512], axis=X)

        def _bounds():
            _bounds_inner(0, qt_s)
            _bounds_inner(1, kt_s)
            _bounds_tail()

        def _bounds_tail():
            mq = stats.tile([1, 1], F32, tag="mq")
            mk = stats.tile([1, 1], F32, tag="mk")
            nc.vector.reduce_max(mq[:], gmax[0:1, 0:8], axis=X)
            nc.vector.reduce_max(mk[:], gmax[0:1, 8:16], axis=X)
            nav = stats.tile([1, 1], F32, tag="nav")
            nc.vector.tensor_add(nav[:], mq[:], mk[:])
            nc.vector.tensor_scalar(nav[:], nav[:], -SCALE / 2.0,
                                    -SCALE * maxf, op0=ALU.mult, op1=ALU.add)
            nap = psum.tile([P, 1024], F32, tag="ps")
            nc.tensor.matmul(nap[0:P, 0:1], onesrow[:], nav[:],
                             start=True, stop=True)
            nc.scalar.copy(neg_a[:], nap[0:P, 0:1])

        _proj(xq_d, wq_s, qt_s)
        _proj(xk_d, wk_s, kt_s)
        _bounds()          # overlaps the V projection below
        _proj(xv_d, wv_s, vt_s)
        _v_transpose()

        # ---- build augmented per-head Q/K operands ----
        # Even heads sit at partitions 0:64 already (DVE copy); odd heads
        # need a partition shift (SBUF->SBUF DMA). Row 64: ones / c_j.
        for pair in range(2):
            for hh in range(2):
                h = pair * 2 + hh
                if hh == 0:
                    nc.vector.tensor_copy(qa_s[h][0:64, :],
                                          qt_s[0:64, pair, :])
                    nc.vector.tensor_copy(ka_s[h][0:64, :],
                                          kt_s[0:64, pair, :])
                else:
                    nc.sync.dma_start(qa_s[h][0:64, :], qt_s[64:P, pair, :])
                    nc.sync.dma_start(ka_s[h][0:64, :], kt_s[64:P, pair, :])
                nc.any.memset(qa_s[h][64:65, :], 1.0)
                nc.sync.dma_start(ka_s[h][64:65, :], c_d[0:1, :])

        rowsum = stats.tile([P, H_LOC, 2 * ICH], F32, tag="rowsum")
        rinv = stats.tile([P, H_LOC, ICH], F32, tag="rinv")
        ot_sb = [opool.tile([P, T], BF16, tag=f"ot{p}", name=f"ot{p}")
                 for p in range(2)]

        # ---- main loop ----
        for sl in range(SL):
            pt_t = [ptpool.tile([P, IC_PER_SL, JCH, P], BF16, tag=f"pt{h}",
                                name=f"pt{h}_{sl}") for h in range(H_LOC)]
            for icm in range(IC_PER_SL):
                ic = sl * IC_PER_SL + icm
                for pair in range(2):
                    ph = [phpool.tile([P, T], BF16, tag="ph",
                                      name=f"ph{ic}_{pair}{i2}") for i2 in range(2)]
                    for half in range(2):
                        j0 = half * 1024
                        sp = [psum.tile([P, 1024], F32, tag="ps",
                                        name=f"sp{ic}_{pair}{half}{i2}")
                              for i2 in range(2)]
                        # fused S + c_j: K=65 matmuls, hh-major so the two
                        # nb matmuls of a head share the stationary operand
                        for hh in range(2):
                            h = pair * 2 + hh
                            for nb in range(2):
                                nc.tensor.matmul(
                                    sp[hh][:, nb * 512:(nb + 1) * 512],
                                    qa_s[h][:, ic * P:(ic + 1) * P],
                                    ka_s[h][:, j0 + nb * 512:
                                            j0 + (nb + 1) * 512],
                                    start=True, stop=True)
                        for hh in range(2):
                            h = pair * 2 + hh
                            nc.scalar.activation(
                                ph[hh][:, j0:j0 + 1024], sp[hh][:],
                                AF.Exp, bias=neg_a[:], scale=SCALE,
                                accum_out=rowsum[:, h,
                                                 2 * ic + half:2 * ic + half + 1])
                    for hh in range(2):
                        h = pair * 2 + hh
                        nc.vector.tensor_add(
                            rinv[:, h, ic:ic + 1],
                            rowsum[:, h, 2 * ic:2 * ic + 1],
                            rowsum[:, h, 2 * ic + 1:2 * ic + 2])
                        nc.vector.reciprocal(rinv[:, h, ic:ic + 1],
                                             rinv[:, h, ic:ic + 1])
                        nc.vector.tensor_scalar_mul(ph[hh][:], ph[hh][:],
                                                    rinv[:, h, ic:ic + 1])
                        nc.sync.dma_start_transpose(out=pt_t[h][:, icm],
                                                    in_=ph[hh][:])
            # PV: O^T[d_pair, i_slice] accumulated over j chunks.
            # Heads of a pair col-tile the array concurrently; each head
            # accumulates in its own PSUM bank (A rows 0:64, B rows 64:128).
            for pair in range(2):
                opA = pvps.tile([P, 512], F32, tag="pv", name=f"opA{sl}{pair}")
                opB = pvps.tile([P, 512], F32, tag="pv", name=f"opB{sl}{pair}")
                for jc in range(JCH):
                    for hh, op in ((0, opA), (1, opB)):
                        h = pair * 2 + hh
                        nc.tensor.matmul(
                            op[hh * 64:(hh + 1) * 64, :],
                            v_s[:, jc, pair * P + hh * 64:
                                pair * P + (hh + 1) * 64],
                            pt_t[h][:, :, jc, :],
                            start=(jc == 0), stop=(jc == JCH - 1),
                            tile_position=(0, 64 * hh))
                nc.vector.tensor_copy(
                    ot_sb[pair][0:64, sl * 512:(sl + 1) * 512], opA[0:64, :])
                nc.vector.tensor_copy(
                    ot_sb[pair][64:P, sl * 512:(sl + 1) * 512], opB[64:P, :])

            # ---- out projection for this slice's t-blocks ----
            for tb in range(sl * IC_PER_SL, (sl + 1) * IC_PER_SL):
                ops = psum.tile([P, 1024], F32, tag="ps", name=f"op{tb}")
                for cc in range(2):
                    lhsT = ot_sb[cc][:, tb * P:(tb + 1) * P]
                    for nb in range(2):
                        nc.tensor.matmul(
                            ops[:, nb * 512:(nb + 1) * 512], lhsT,
                            wo_s[:, cc, nb * 512:(nb + 1) * 512],
                            start=(cc == 0), stop=(cc == 1))
                ostage = opool.tile([P, D], F32, tag="ostage")
                nc.vector.tensor_copy(ostage[:], ops[:])
                nc.gpsimd.dma_start(out_d[tb * P:(tb + 1) * P, :], ostage[:])

    nc.compile()
    return nc


_last_results = None


def _host_cvec(frac: np.ndarray, bs: float):
    """Per-column logit bias c_j = -bs/f_j (the rank-1 collapse of the F
    matrix; the row term bs/f_i cancels in softmax). Centered to max 0 and
    replicated across all 128 partitions for the K=1 ones-matmul rhs."""
    cvecs = []
    for b in range(B):
        f = frac[b].astype(np.float64)
        c = -bs / f
        c = c - c.max()
        cm = np.broadcast_to(c.astype(ml_dtypes.bfloat16), (P, T))
        cvecs.append(np.ascontiguousarray(cm))
    return cvecs


def _prepare(inputs):
    """Build the program and per-core input maps from full inputs."""
    inp = {k: np.asarray(v) for k, v in inputs.items()}
    query, key, value = inp["query"], inp["key"], inp["value"]
    frac = inp["frac"]
    Wq, Wk, Wv, Wo = inp["Wq"], inp["Wk"], inp["Wv"], inp["Wo"]
    attn_bias = inp["attn_bias"]

    bs = float(np.sum(attn_bias.astype(np.float64)))
    cvecs = _host_cvec(frac, bs)
    # c is centered (max 0); keep a small positive margin in the bound.
    maxf = 1.0

    nc = _build_program(maxf)

    in_maps = []
    for c in range(N_CORES):
        b, g = c // H_LOC, c % H_LOC
        sl = slice(g * C_LOC, (g + 1) * C_LOC)
        in_maps.append({
            "xq": np.ascontiguousarray(query[b].T).astype(ml_dtypes.bfloat16),
            "xk": np.ascontiguousarray(key[b].T).astype(ml_dtypes.bfloat16),
            "xv": np.ascontiguousarray(value[b].T).astype(ml_dtypes.bfloat16),
            "wq": np.ascontiguousarray(Wq[sl, :].T).astype(ml_dtypes.bfloat16),
            "wk": np.ascontiguousarray(Wk[sl, :].T).astype(ml_dtypes.bfloat16),
            "wv": np.ascontiguousarray(Wv[sl, :].T).astype(ml_dtypes.bfloat16),
            "wo": np.ascontiguousarray(Wo[:, sl].T).astype(ml_dtypes.bfloat16),
            "cvec": cvecs[b],
        })
    return nc, in_maps


def kernel(**inputs) -> np.ndarray:
    nc, in_maps = _prepare(inputs)

    res = run_bass_kernel_spmd(nc, in_maps, list(range(N_CORES)))
    global _last_results
    _last_results = res

    out = np.zeros((B, T, D), dtype=np.float32)
    for c in range(N_CORES):
        out[c // H_LOC] += np.asarray(res.results[c]["out"])
    out += np.asarray(inputs["bo"], dtype=np.float32)[None, None, :]
    return out
